# revision 23
# baseline (speedup 1.0000x reference)
"""Multi-head attention (B=4, S=2048, D=1024, 16 heads x 64) on 8 NeuronCores.

Sharding: DP=4 over batch x TP=2 over heads (8 heads/core).
Each core computes, for one batch element and half the heads:
    qhT = (q @ Wq + bq)^T       [512, 2048]   (bf16, head-dim on partitions)
    khT = (k @ Wk + bk)^T       [512, 2048]
    vh  = v @ Wv                [2048, 512]   (natural layout, k on partitions)
    per (head-pair, q-chunk): flash-style S^T = kh @ qh^T, p = exp(scale*S^T),
      outT_u = vh^T @ p (col-tiled pair) and l = ones^T @ p (replicated rows),
      outT = outT_u * approx_recip(l)
    partial_out = outT^T @ Wo_shard          [2048, 1024]  (fp32)
Host sums the TP pair partials and adds the bias terms (bv @ Wo + bo).

v3 schedule: bulk DMA (one descriptor per tensor slab), attention starts as
soon as kT/qT-slice-0 land; v/k/q projection chains and the fc output
projection ride as deadline-paced fillers inside the attention iterations;
hp=2/hp=3 blocks interleave so fc work spreads instead of cliffing at the
end.  All PSUM drains off the Scalar engine (it only runs exp).
"""

import os
import sys

sys.path.insert(0, "/opt/trn_rl_repo")

import numpy as np
import ml_dtypes

S = 2048          # sequence length
DM = 1024         # model dim
HD = 512          # local head-dim total (8 heads x 64) per core (TP=2)
NB = 4            # batch
NCORES = 8
P = 128
DK = 64
SCALE = 1.0 / 8.0  # 1/sqrt(64)

NM = DM // P      # 8 m-chunks
NHP = HD // P     # 4 head pairs
NSC = S // 512    # 4 s-chunks of 512
NJ = S // P       # 16 k-chunks
NSL = 4           # input column slices of 512

_CACHE = {}


def _build_nc():
    import concourse.bass as bass  # noqa: F401
    import concourse.mybir as mybir
    from concourse import bacc, tile
    from contextlib import ExitStack

    BF = mybir.dt.bfloat16
    F32 = mybir.dt.float32
    Exp = mybir.ActivationFunctionType.Exp

    nc = bacc.Bacc("TRN2", target_bir_lowering=False, debug=False, num_swdge_queues=4)

    # DRAM inputs, pre-slabbed on the host into the exact SBUF layout
    # ([128 partitions, m-major columns]) so every DMA is one contiguous
    # full-rate 2D copy.
    kTa = nc.dram_tensor("kTa", [P, 4 * S], BF, kind="ExternalInput")
    kTb = nc.dram_tensor("kTb", [P, 4 * S], BF, kind="ExternalInput")
    qTs = [nc.dram_tensor(f"qTs{s}", [P, NM * 512], BF, kind="ExternalInput") for s in range(NSL)]
    vTs = [nc.dram_tensor(f"vTs{s}", [P, NM * P], BF, kind="ExternalInput") for s in range(NJ)]
    wq = nc.dram_tensor("wq", [P, NM * HD], BF, kind="ExternalInput")
    wk = nc.dram_tensor("wk", [P, NM * HD], BF, kind="ExternalInput")
    wv = nc.dram_tensor("wv", [P, NM * HD], BF, kind="ExternalInput")
    wo = nc.dram_tensor("wo", [P, NHP * DM], BF, kind="ExternalInput")
    bq = nc.dram_tensor("bq", [HD], F32, kind="ExternalInput")
    bk = nc.dram_tensor("bk", [HD], F32, kind="ExternalInput")
    out = nc.dram_tensor("out", [S, DM], F32, kind="ExternalOutput")

    with ExitStack() as ctx:
        tc = ctx.enter_context(tile.TileContext(nc))

        const = ctx.enter_context(tc.tile_pool(name="const", bufs=1))
        w_pool = ctx.enter_context(tc.tile_pool(name="w_pool", bufs=4))
        kt_pool = ctx.enter_context(tc.tile_pool(name="kt_pool", bufs=2))
        qt_pool = ctx.enter_context(tc.tile_pool(name="qt_pool", bufs=3))
        vt_pool = ctx.enter_context(tc.tile_pool(name="vt_pool", bufs=4))
        qh_pool = ctx.enter_context(tc.tile_pool(name="qh_pool", bufs=4))
        kh_pool = ctx.enter_context(tc.tile_pool(name="kh_pool", bufs=4))
        vh_pool = ctx.enter_context(tc.tile_pool(name="vh_pool", bufs=16))
        outT_pool = ctx.enter_context(tc.tile_pool(name="outT_pool", bufs=4))
        p_pool = ctx.enter_context(tc.tile_pool(name="p_pool", bufs=4))
        rec_pool = ctx.enter_context(tc.tile_pool(name="rec_pool", bufs=1))
        stage_pool = ctx.enter_context(tc.tile_pool(name="stage_pool", bufs=2))
        st_ps = ctx.enter_context(tc.tile_pool(name="st_ps", bufs=2, space="PSUM"))
        pv_ps = ctx.enter_context(tc.tile_pool(name="pv_ps", bufs=2, space="PSUM"))
        wk_ps = ctx.enter_context(tc.tile_pool(name="wk_ps", bufs=2, space="PSUM"))

        # constants (gpsimd queue)
        ones_t = const.tile([P, DK], BF, tag="ones")
        nc.vector.memset(ones_t[:], 1.0)
        bq_sb = const.tile([P, NHP], F32, tag="bq")
        nc.gpsimd.dma_start(bq_sb[:], bq[:].rearrange("(f p) -> p f", p=P))
        bk_sb = const.tile([P, NHP], F32, tag="bk")
        nc.gpsimd.dma_start(bk_sb[:], bk[:].rearrange("(f p) -> p f", p=P))

        # ---- bulk input DMA, priority-ordered on the sync queue ----
        # Slabs are host-pre-arranged: each is one contiguous 2D copy.
        def load_slab(pool, handle, tag):
            t = pool.tile([P, handle.shape[1]], BF, tag=tag)
            nc.sync.dma_start(t[:], handle[:, :])
            return t

        wk_sb = load_slab(w_pool, wk, "w")       # [128, 8*512]
        kT_a = load_slab(kt_pool, kTa, "kt")
        kT_b = load_slab(kt_pool, kTb, "kt")
        wq_sb = load_slab(w_pool, wq, "w")
        qT_sb = [None] * NSL
        vT_sb = [None] * NJ
        qT_sb[0] = load_slab(qt_pool, qTs[0], "qt")   # [128, 8*512]
        wv_sb = load_slab(w_pool, wv, "w")
        for j in range(4):
            vT_sb[j] = load_slab(vt_pool, vTs[j], "vt")
        qT_sb[1] = load_slab(qt_pool, qTs[1], "qt")
        for j in range(4, 8):
            vT_sb[j] = load_slab(vt_pool, vTs[j], "vt")
        qT_sb[2] = load_slab(qt_pool, qTs[2], "qt")
        for j in range(8, NJ):
            vT_sb[j] = load_slab(vt_pool, vTs[j], "vt")
        qT_sb[3] = load_slab(qt_pool, qTs[3], "qt")
        wo_sb = load_slab(w_pool, wo, "wo")      # [128, 4*1024]

        # ---- PE warm-up: keep the HAM clock gate open during the DMA
        # prefix so the first projection chains run at 2.4 GHz ----
        warm_p = p_pool.tile([P, 1024], BF, tag="p", name="warm")
        nc.vector.memset(warm_p[:], 0.0)
        wps = wk_ps.tile([P, HD], F32, tag="wps", name="warm_ps")
        for _ in range(38):
            nc.tensor.matmul(
                wps[0:64, :], lhsT=ones_t[:], rhs=warm_p[:, 0:512], start=True, stop=True
            )

        def w_sl(t, m, lo, hi, w=512):
            return t[:, m * w + lo : m * w + hi]

        def kt_sl(m, lo, hi):
            t = kT_a if m < 4 else kT_b
            return t[:, (m % 4) * S + lo : (m % 4) * S + hi]

        # ---- persistent activation tiles ----
        qhT_sb = [qh_pool.tile([P, S], BF, tag="qh", name=f"qhT{i}") for i in range(NHP)]
        khT_sb = [kh_pool.tile([P, S], BF, tag="kh", name=f"khT{i}") for i in range(NHP)]
        outT_sb = [outT_pool.tile([P, S], BF, tag="outT", name=f"outT{i}") for i in range(NHP)]
        vh_sb = [vh_pool.tile([P, HD], BF, tag="vh", name=f"vh{i}") for i in range(NJ)]

        # ---- chain builders (lists of single-op closures) ----
        def vproj_chain_ops(sc):
            """vh[sc] = vT[:, sc-chunk]^T @ wv   (natural [s, hd] layout)."""
            cell = {}

            def mk(m):
                def op():
                    if m == 0:
                        cell["ps"] = wk_ps.tile([P, HD], F32, tag="wps", name="vps")
                    nc.tensor.matmul(
                        cell["ps"][:],
                        lhsT=w_sl(vT_sb[sc], m, 0, P, w=P),
                        rhs=w_sl(wv_sb, m, 0, HD),
                        start=(m == 0),
                        stop=(m == NM - 1),
                    )
                return op

            ops = [mk(m) for m in range(NM)]
            ops.append(lambda: nc.vector.tensor_copy(vh_sb[sc][:], cell["ps"][:]))
            return ops

        def qproj_chain_ops(hp, sc):
            cell = {}

            def mk(m):
                def op():
                    if m == 0:
                        cell["ps"] = wk_ps.tile([P, HD], F32, tag="wps", name="qps")
                    nc.tensor.matmul(
                        cell["ps"][:],
                        lhsT=w_sl(wq_sb, m, hp * P, (hp + 1) * P),
                        rhs=w_sl(qT_sb[sc], m, 0, 512),
                        start=(m == 0),
                        stop=(m == NM - 1),
                    )
                return op

            ops = [mk(m) for m in range(NM)]

            def ev():
                nc.vector.tensor_scalar_add(
                    qhT_sb[hp][:, sc * 512 : (sc + 1) * 512],
                    cell["ps"][:],
                    bq_sb[:, hp : hp + 1],
                )

            ops.append(ev)
            return ops

        def kproj_chain_ops(hp, sc):
            cell = {}

            def mk(m):
                def op():
                    if m == 0:
                        cell["ps"] = wk_ps.tile([P, HD], F32, tag="wps", name="kps")
                    nc.tensor.matmul(
                        cell["ps"][:],
                        lhsT=w_sl(wk_sb, m, hp * P, (hp + 1) * P),
                        rhs=kt_sl(m, sc * 512, (sc + 1) * 512),
                        start=(m == 0),
                        stop=(m == NM - 1),
                    )
                return op

            ops = [mk(m) for m in range(NM)]

            def ev():
                nc.vector.tensor_scalar_add(
                    khT_sb[hp][:, sc * 512 : (sc + 1) * 512],
                    cell["ps"][:],
                    bk_sb[:, hp : hp + 1],
                )

            ops.append(ev)
            return ops

        def fc_chain_ops(sc, ec, pool=None):
            ss = slice(sc * P, (sc + 1) * P)
            cell = {}

            def mk(hp):
                def op():
                    if hp == 0:
                        pl = pool or wk_ps
                        cell["ps"] = pl.tile(
                            [P, 1024 if pl is st_ps else HD], F32,
                            tag="stps" if pl is st_ps else "wps", name="fps",
                        )[:, 0:HD]
                    nc.tensor.matmul(
                        cell["ps"][:],
                        lhsT=outT_sb[hp][:, ss],
                        rhs=w_sl(wo_sb, hp, ec * 512, (ec + 1) * 512, w=DM),
                        start=(hp == 0),
                        stop=(hp == NHP - 1),
                    )
                return op

            ops = [mk(hp) for hp in range(NHP)]

            def ev():
                stg = stage_pool.tile([P, 512], F32, tag="stg", name="stg")
                nc.vector.tensor_copy(stg[:], cell["ps"][:])
                nc.gpsimd.dma_start(out[ss, ec * 512 : (ec + 1) * 512], stg[:])

            ops.append(ev)
            return ops

        # ---- filler queue with deadline pacing ----
        from collections import deque

        fillers = deque()   # flat op deque
        drained = [0]       # ops popped so far
        emitted = [0]       # ops pushed so far
        checkpoints = []    # (global_iter, cum_ops needed by then)
        debt = [0.0]

        def push_chain(ops):
            fillers.extend(ops)
            emitted[0] += len(ops)
            return emitted[0]

        def drain(n):
            for _ in range(n):
                if not fillers:
                    return
                fillers.popleft()()
                drained[0] += 1

        def drain_until(cum):
            while drained[0] < cum and fillers:
                fillers.popleft()()
                drained[0] += 1

        # block order: hp 0 and 1 sweep qc; then hp2/hp3 interleave per qc so
        # fc(qc) (which needs all four hp at qc) unlocks early and spreads.
        BLOCKS = (
            [(0, qc) for qc in range(NSC)]
            + [(1, qc) for qc in range(NSC)]
            + [b for qc in range(NSC) for b in ((2, qc), (3, qc))]
        )
        block_gi = {b: 16 * i for i, b in enumerate(BLOCKS)}

        # filler order: v chains first (PV gating), then per-hp k/q proj in
        # first-use order.
        v_need = {}
        # kp0 sc1-3: needed by st(j>=4) of block (0,0) - fine-grained deadlines
        for sc in range(1, NSC):
            checkpoints.append((4 * sc, push_chain(kproj_chain_ops(0, sc))))
        for j in range(8):
            v_need[j] = push_chain(vproj_chain_ops(j))
        # qp(0,1) rides mid-v so its iter-16 deadline doesn't force a burst
        checkpoints.append((block_gi[(0, 1)], push_chain(qproj_chain_ops(0, 1))))
        for j in range(8, NJ):
            v_need[j] = push_chain(vproj_chain_ops(j))
        checkpoints.append((block_gi[(0, 2)], push_chain(qproj_chain_ops(0, 2))))
        # qp(hp,0) early so qTs0's buffer frees for qTs3
        for hp in range(1, NHP):
            checkpoints.append((block_gi[(hp, 0)], push_chain(qproj_chain_ops(hp, 0))))
        checkpoints.append((block_gi[(0, 3)], push_chain(qproj_chain_ops(0, 3))))
        for hp in range(1, NHP):
            for sc in range(NSC):
                cp = push_chain(kproj_chain_ops(hp, sc))
            checkpoints.append((block_gi[(hp, 0)], cp))
            for sc in range(1, NSC):
                checkpoints.append((block_gi[(hp, sc)], push_chain(qproj_chain_ops(hp, sc))))
        FC_DEADLINE = 252

        def pace(gi):
            """Drain fillers: meet every checkpoint, else eagerly ~2/iter."""
            need = 0.0
            for it, cum in checkpoints:
                if it <= gi:
                    drain_until(cum)
                elif cum > drained[0]:
                    need = max(need, (cum - drained[0]) / (it - gi))
            backlog = emitted[0] - drained[0]
            if backlog > 0:
                need = max(
                    need,
                    backlog / max(1.0, FC_DEADLINE - gi),
                    min(2.0, float(backlog)),
                )
            debt[0] += need
            n = int(debt[0])
            if n > 0:
                drain(n)
                debt[0] -= n

        # ---- upfront: only what the first 4 iterations need ----
        for op in kproj_chain_ops(0, 0):
            op()
        for op in qproj_chain_ops(0, 0):
            op()

        # ---- attention blocks ----
        carry = []
        LAG = 3

        def attn_block(hp, qc, gi0):
            qs = slice(qc * 512, (qc + 1) * 512)
            state = {}
            p_tiles = {}

            def emit_st(j):
                ks = slice(j * P, (j + 1) * P)
                st = st_ps.tile([P, 1024], F32, tag="stps")
                nc.tensor.matmul(
                    st[:, 0:512],
                    lhsT=khT_sb[hp][0:64, ks],
                    rhs=qhT_sb[hp][0:64, qs],
                    start=True,
                    stop=True,
                    tile_position=(0, 0),
                )
                nc.tensor.matmul(
                    st[:, 512:1024],
                    lhsT=khT_sb[hp][64:128, ks],
                    rhs=qhT_sb[hp][64:128, qs],
                    start=True,
                    stop=True,
                    tile_position=(64, 0),
                )
                p = p_pool.tile([P, 1024], BF, tag="p")
                nc.scalar.activation(p[:], st[:], Exp, scale=SCALE)
                p_tiles[j] = p

            def emit_pv(j):
                if "P" not in state:
                    state["P"] = pv_ps.tile([P, 512], F32, tag="pvps", name="Pps")
                    state["L"] = pv_ps.tile([P, 512], F32, tag="pvps", name="Lps")
                P_ps, L_ps = state["P"], state["L"]
                p = p_tiles.pop(j)
                first, last = (j == 0), (j == NJ - 1)
                nc.tensor.matmul(
                    P_ps[0:64, :],
                    lhsT=vh_sb[j][:, hp * P : hp * P + DK],
                    rhs=p[:, 0:512],
                    start=first,
                    stop=last,
                    tile_position=(0, 0),
                    skip_group_check=True,
                )
                nc.tensor.matmul(
                    P_ps[64:128, :],
                    lhsT=vh_sb[j][:, hp * P + DK : (hp + 1) * P],
                    rhs=p[:, 512:1024],
                    start=first,
                    stop=last,
                    tile_position=(0, 64),
                    skip_group_check=True,
                )
                nc.tensor.matmul(
                    L_ps[0:64, :],
                    lhsT=ones_t[:],
                    rhs=p[:, 0:512],
                    start=first,
                    stop=last,
                    tile_position=(0, 0),
                    skip_group_check=True,
                )
                nc.tensor.matmul(
                    L_ps[64:128, :],
                    lhsT=ones_t[:],
                    rhs=p[:, 512:1024],
                    start=first,
                    stop=last,
                    tile_position=(0, 64),
                    skip_group_check=True,
                )

            for j in range(NJ):
                emit_st(j)
                if carry:
                    carry.pop(0)()
                if j == 4:
                    # prev block's carry (incl. normalize) has emitted by now,
                    # so fc chains reading its outT are safe to enqueue
                    while pending_fc:
                        push_chain(pending_fc.pop(0))
                if j >= LAG:
                    jj = j - LAG
                    drain_until(v_need.get(jj, 0))
                    emit_pv(jj)
                pace(gi0 + j)

            def mk_pv(j):
                def op():
                    drain_until(v_need.get(j, 0))
                    emit_pv(j)
                return op

            def normalize():
                rec = rec_pool.tile([P, 512], F32, tag="rec")
                nc.vector.reciprocal_approx_fast(rec[:], state["L"][:])
                nc.vector.tensor_mul(outT_sb[hp][:, qs], state["P"][:], rec[:])

            return [mk_pv(j) for j in range(NJ - LAG, NJ)] + [normalize]

        pending_fc = []
        for bi, (hp, qc) in enumerate(BLOCKS):
            carry = attn_block(hp, qc, 16 * bi)
            if hp == NHP - 1:
                last = bi == len(BLOCKS) - 1
                for i, sc in enumerate(range(qc * 4, qc * 4 + 4)):
                    for ec in range(2):
                        # tail batch alternates into the now-idle st_ps pool
                        # so chains pipeline two-deep
                        pool = st_ps if (last and (i * 2 + ec) % 2) else None
                        pending_fc.append(fc_chain_ops(sc, ec, pool))
        for op in carry:
            op()

        while pending_fc:
            fillers.extend(pending_fc.pop(0))
        while fillers:
            fillers.popleft()()
            drained[0] += 1

    nc.compile()
    return nc


def _get_nc():
    if "nc" not in _CACHE:
        _CACHE["nc"] = _build_nc()
    return _CACHE["nc"]


def kernel(q, k, v, Wq, bq, Wk, bk, Wv, bv, Wo, bo):
    from concourse.bass_utils import run_bass_kernel_spmd

    bf16 = ml_dtypes.bfloat16
    q, k, v = (np.asarray(x, np.float32) for x in (q, k, v))
    Wq, bq, Wk, bk, Wv, bv, Wo, bo = (
        np.asarray(x, np.float32) for x in (Wq, bq, Wk, bk, Wv, bv, Wo, bo)
    )

    def slab(a):
        """(n*128, C) -> (128, n*C): partition-major, m-chunk-major columns."""
        R, C = a.shape
        n = R // 128
        return np.ascontiguousarray(
            a.reshape(n, 128, C).transpose(1, 0, 2).reshape(128, n * C)
        )

    in_maps = []
    for c in range(NCORES):
        b, t = c // 2, c % 2
        hs = slice(t * HD, (t + 1) * HD)
        qT = q[b].T.astype(bf16)
        kT = k[b].T.astype(bf16)
        vT = v[b].T.astype(bf16)
        im = {
            "kTa": slab(kT[0 : DM // 2]),
            "kTb": slab(kT[DM // 2 :]),
            "wq": slab(Wq[:, hs].astype(bf16)),
            "wk": slab(Wk[:, hs].astype(bf16)),
            "wv": slab(Wv[:, hs].astype(bf16)),
            "wo": slab(Wo[hs, :].astype(bf16)),
            "bq": np.ascontiguousarray(bq[hs]),
            "bk": np.ascontiguousarray(bk[hs]),
        }
        for s in range(NSL):
            im[f"qTs{s}"] = slab(qT[:, s * 512 : (s + 1) * 512])
        for j in range(NJ):
            im[f"vTs{j}"] = slab(vT[:, j * 128 : (j + 1) * 128])
        in_maps.append(im)

    nc = _get_nc()
    trace = os.environ.get("KERNEL_TRACE", "0") == "1"
    res = run_bass_kernel_spmd(
        nc, in_maps, core_ids=list(range(NCORES)), trace=trace
    )
    if trace:
        print(f"HW exec time: {res.exec_time_ns} ns")

    host_bias = (bv @ Wo + bo).astype(np.float32)
    full = np.empty((NB, S, DM), np.float32)
    for b in range(NB):
        full[b] = res.results[2 * b]["out"] + res.results[2 * b + 1]["out"] + host_bias
    return full


# revision 24
# speedup vs baseline: 1.0156x; 1.0156x over previous
"""Multi-head attention (B=4, S=2048, D=1024, 16 heads x 64) on 8 NeuronCores.

Sharding: DP=4 over batch x TP=2 over heads (8 heads/core).
Each core computes, for one batch element and half the heads:
    qhT = (q @ Wq + bq)^T       [512, 2048]   (bf16, head-dim on partitions)
    khT = (k @ Wk + bk)^T       [512, 2048]
    vh  = v @ Wv                [2048, 512]   (natural layout, k on partitions)
    per (head-pair, q-chunk): flash-style S^T = kh @ qh^T, p = exp(scale*S^T),
      outT_u = vh^T @ p (col-tiled pair) and l = ones^T @ p (replicated rows),
      outT = outT_u * approx_recip(l)
    partial_out = outT^T @ Wo_shard          [2048, 1024]  (fp32)
Host sums the TP pair partials and adds the bias terms (bv @ Wo + bo).

v3 schedule: bulk DMA (one descriptor per tensor slab), attention starts as
soon as kT/qT-slice-0 land; v/k/q projection chains and the fc output
projection ride as deadline-paced fillers inside the attention iterations;
hp=2/hp=3 blocks interleave so fc work spreads instead of cliffing at the
end.  All PSUM drains off the Scalar engine (it only runs exp).
"""

import os
import sys

sys.path.insert(0, "/opt/trn_rl_repo")

import numpy as np
import ml_dtypes

S = 2048          # sequence length
DM = 1024         # model dim
HD = 512          # local head-dim total (8 heads x 64) per core (TP=2)
NB = 4            # batch
NCORES = 8
P = 128
DK = 64
SCALE = 1.0 / 8.0  # 1/sqrt(64)

NM = DM // P      # 8 m-chunks
NHP = HD // P     # 4 head pairs
NSC = S // 512    # 4 s-chunks of 512
NJ = S // P       # 16 k-chunks
NSL = 4           # input column slices of 512

_CACHE = {}


def _build_nc():
    import concourse.bass as bass  # noqa: F401
    import concourse.mybir as mybir
    from concourse import bacc, tile
    from contextlib import ExitStack

    BF = mybir.dt.bfloat16
    F32 = mybir.dt.float32
    Exp = mybir.ActivationFunctionType.Exp

    nc = bacc.Bacc("TRN2", target_bir_lowering=False, debug=False, num_swdge_queues=4)

    # DRAM inputs, pre-slabbed on the host into the exact SBUF layout
    # ([128 partitions, m-major columns]) so every DMA is one contiguous
    # full-rate 2D copy.
    kTa = nc.dram_tensor("kTa", [P, 4 * S], BF, kind="ExternalInput")
    kTb = nc.dram_tensor("kTb", [P, 4 * S], BF, kind="ExternalInput")
    qTs = [nc.dram_tensor(f"qTs{s}", [P, NM * 512], BF, kind="ExternalInput") for s in range(NSL)]
    vTs = [nc.dram_tensor(f"vTs{s}", [P, NM * P], BF, kind="ExternalInput") for s in range(NJ)]
    wq = nc.dram_tensor("wq", [P, NM * HD], BF, kind="ExternalInput")
    wk = nc.dram_tensor("wk", [P, NM * HD], BF, kind="ExternalInput")
    wv = nc.dram_tensor("wv", [P, NM * HD], BF, kind="ExternalInput")
    wo = nc.dram_tensor("wo", [P, NHP * DM], BF, kind="ExternalInput")
    bq = nc.dram_tensor("bq", [HD], F32, kind="ExternalInput")
    bk = nc.dram_tensor("bk", [HD], F32, kind="ExternalInput")
    out = nc.dram_tensor("out", [S, DM], F32, kind="ExternalOutput")

    with ExitStack() as ctx:
        tc = ctx.enter_context(tile.TileContext(nc))

        const = ctx.enter_context(tc.tile_pool(name="const", bufs=1))
        w_pool = ctx.enter_context(tc.tile_pool(name="w_pool", bufs=4))
        kt_pool = ctx.enter_context(tc.tile_pool(name="kt_pool", bufs=2))
        qt_pool = ctx.enter_context(tc.tile_pool(name="qt_pool", bufs=3))
        vt_pool = ctx.enter_context(tc.tile_pool(name="vt_pool", bufs=4))
        qh_pool = ctx.enter_context(tc.tile_pool(name="qh_pool", bufs=4))
        kh_pool = ctx.enter_context(tc.tile_pool(name="kh_pool", bufs=4))
        vh_pool = ctx.enter_context(tc.tile_pool(name="vh_pool", bufs=16))
        outT_pool = ctx.enter_context(tc.tile_pool(name="outT_pool", bufs=4))
        p_pool = ctx.enter_context(tc.tile_pool(name="p_pool", bufs=4))
        rec_pool = ctx.enter_context(tc.tile_pool(name="rec_pool", bufs=1))
        stage_pool = ctx.enter_context(tc.tile_pool(name="stage_pool", bufs=2))
        st_ps = ctx.enter_context(tc.tile_pool(name="st_ps", bufs=2, space="PSUM"))
        pv_ps = ctx.enter_context(tc.tile_pool(name="pv_ps", bufs=2, space="PSUM"))
        wk_ps = ctx.enter_context(tc.tile_pool(name="wk_ps", bufs=2, space="PSUM"))

        # constants (gpsimd queue)
        ones_t = const.tile([P, DK], BF, tag="ones")
        nc.vector.memset(ones_t[:], 1.0)
        bq_sb = const.tile([P, NHP], F32, tag="bq")
        nc.gpsimd.dma_start(bq_sb[:], bq[:].rearrange("(f p) -> p f", p=P))
        bk_sb = const.tile([P, NHP], F32, tag="bk")
        nc.gpsimd.dma_start(bk_sb[:], bk[:].rearrange("(f p) -> p f", p=P))

        # ---- bulk input DMA, priority-ordered on the sync queue ----
        # Slabs are host-pre-arranged: each is one contiguous 2D copy.
        def load_slab(pool, handle, tag):
            t = pool.tile([P, handle.shape[1]], BF, tag=tag)
            nc.sync.dma_start(t[:], handle[:, :])
            return t

        wk_sb = load_slab(w_pool, wk, "w")       # [128, 8*512]
        kT_a = load_slab(kt_pool, kTa, "kt")
        kT_b = load_slab(kt_pool, kTb, "kt")
        wq_sb = load_slab(w_pool, wq, "w")
        qT_sb = [None] * NSL
        vT_sb = [None] * NJ
        qT_sb[0] = load_slab(qt_pool, qTs[0], "qt")   # [128, 8*512]
        wv_sb = load_slab(w_pool, wv, "w")
        for j in range(4):
            vT_sb[j] = load_slab(vt_pool, vTs[j], "vt")
        qT_sb[1] = load_slab(qt_pool, qTs[1], "qt")
        for j in range(4, 8):
            vT_sb[j] = load_slab(vt_pool, vTs[j], "vt")
        qT_sb[2] = load_slab(qt_pool, qTs[2], "qt")
        for j in range(8, NJ):
            vT_sb[j] = load_slab(vt_pool, vTs[j], "vt")
        qT_sb[3] = load_slab(qt_pool, qTs[3], "qt")
        wo_sb = load_slab(w_pool, wo, "wo")      # [128, 4*1024]

        # ---- PE warm-up: keep the HAM clock gate open during the DMA
        # prefix so the first projection chains run at 2.4 GHz ----
        warm_p = p_pool.tile([P, 1024], BF, tag="p", name="warm")
        nc.vector.memset(warm_p[:], 0.0)
        wps = wk_ps.tile([P, HD], F32, tag="wps", name="warm_ps")
        for _ in range(38):
            nc.tensor.matmul(
                wps[0:64, :], lhsT=ones_t[:], rhs=warm_p[:, 0:512], start=True, stop=True
            )

        def w_sl(t, m, lo, hi, w=512):
            return t[:, m * w + lo : m * w + hi]

        def kt_sl(m, lo, hi):
            t = kT_a if m < 4 else kT_b
            return t[:, (m % 4) * S + lo : (m % 4) * S + hi]

        # ---- persistent activation tiles ----
        qhT_sb = [qh_pool.tile([P, S], BF, tag="qh", name=f"qhT{i}") for i in range(NHP)]
        khT_sb = [kh_pool.tile([P, S], BF, tag="kh", name=f"khT{i}") for i in range(NHP)]
        outT_sb = [outT_pool.tile([P, S], BF, tag="outT", name=f"outT{i}") for i in range(NHP)]
        vh_sb = [vh_pool.tile([P, HD], BF, tag="vh", name=f"vh{i}") for i in range(NJ)]

        # ---- chain builders (lists of single-op closures) ----
        def vproj_chain_ops(sc):
            """vh[sc] = vT[:, sc-chunk]^T @ wv   (natural [s, hd] layout)."""
            cell = {}

            def mk(m):
                def op():
                    if m == 0:
                        cell["ps"] = wk_ps.tile([P, HD], F32, tag="wps", name="vps")
                    nc.tensor.matmul(
                        cell["ps"][:],
                        lhsT=w_sl(vT_sb[sc], m, 0, P, w=P),
                        rhs=w_sl(wv_sb, m, 0, HD),
                        start=(m == 0),
                        stop=(m == NM - 1),
                    )
                return op

            ops = [mk(m) for m in range(NM)]
            ops.append(lambda: nc.vector.tensor_copy(vh_sb[sc][:], cell["ps"][:]))
            return ops

        def qproj_chain_ops(hp, sc):
            cell = {}

            def mk(m):
                def op():
                    if m == 0:
                        cell["ps"] = wk_ps.tile([P, HD], F32, tag="wps", name="qps")
                    nc.tensor.matmul(
                        cell["ps"][:],
                        lhsT=w_sl(wq_sb, m, hp * P, (hp + 1) * P),
                        rhs=w_sl(qT_sb[sc], m, 0, 512),
                        start=(m == 0),
                        stop=(m == NM - 1),
                    )
                return op

            ops = [mk(m) for m in range(NM)]

            def ev():
                nc.vector.tensor_scalar_add(
                    qhT_sb[hp][:, sc * 512 : (sc + 1) * 512],
                    cell["ps"][:],
                    bq_sb[:, hp : hp + 1],
                )

            ops.append(ev)
            return ops

        def kproj_chain_ops(hp, sc):
            cell = {}

            def mk(m):
                def op():
                    if m == 0:
                        cell["ps"] = wk_ps.tile([P, HD], F32, tag="wps", name="kps")
                    nc.tensor.matmul(
                        cell["ps"][:],
                        lhsT=w_sl(wk_sb, m, hp * P, (hp + 1) * P),
                        rhs=kt_sl(m, sc * 512, (sc + 1) * 512),
                        start=(m == 0),
                        stop=(m == NM - 1),
                    )
                return op

            ops = [mk(m) for m in range(NM)]

            def ev():
                nc.vector.tensor_scalar_add(
                    khT_sb[hp][:, sc * 512 : (sc + 1) * 512],
                    cell["ps"][:],
                    bk_sb[:, hp : hp + 1],
                )

            ops.append(ev)
            return ops

        def fc_chain_ops(sc, ec, pool=None):
            ss = slice(sc * P, (sc + 1) * P)
            cell = {}

            def mk(hp):
                def op():
                    if hp == 0:
                        pl = pool or wk_ps
                        cell["ps"] = pl.tile(
                            [P, 1024 if pl is st_ps else HD], F32,
                            tag="stps" if pl is st_ps else "wps", name="fps",
                        )[:, 0:HD]
                    nc.tensor.matmul(
                        cell["ps"][:],
                        lhsT=outT_sb[hp][:, ss],
                        rhs=w_sl(wo_sb, hp, ec * 512, (ec + 1) * 512, w=DM),
                        start=(hp == 0),
                        stop=(hp == NHP - 1),
                    )
                return op

            ops = [mk(hp) for hp in range(NHP)]

            def ev():
                stg = stage_pool.tile([P, 512], F32, tag="stg", name="stg")
                nc.vector.tensor_copy(stg[:], cell["ps"][:])
                nc.gpsimd.dma_start(out[ss, ec * 512 : (ec + 1) * 512], stg[:])

            ops.append(ev)
            return ops

        # ---- filler queue with deadline pacing ----
        from collections import deque

        fillers = deque()   # flat op deque
        drained = [0]       # ops popped so far
        emitted = [0]       # ops pushed so far
        checkpoints = []    # (global_iter, cum_ops needed by then)
        debt = [0.0]

        def push_chain(ops):
            fillers.extend(ops)
            emitted[0] += len(ops)
            return emitted[0]

        def drain(n):
            for _ in range(n):
                if not fillers:
                    return
                fillers.popleft()()
                drained[0] += 1

        def drain_until(cum):
            while drained[0] < cum and fillers:
                fillers.popleft()()
                drained[0] += 1

        # block order: hp 0 and 1 sweep qc; then hp2/hp3 interleave per qc so
        # fc(qc) (which needs all four hp at qc) unlocks early and spreads.
        BLOCKS = (
            [(0, qc) for qc in range(NSC)]
            + [(1, qc) for qc in range(NSC)]
            + [b for qc in range(NSC) for b in ((2, qc), (3, qc))]
        )
        block_gi = {b: 16 * i for i, b in enumerate(BLOCKS)}

        # filler order: v chains first (PV gating), then per-hp k/q proj in
        # first-use order.
        v_need = {}
        # kp0 sc1-3: needed by st(j>=4) of block (0,0) - fine-grained deadlines
        for sc in range(1, NSC):
            checkpoints.append((4 * sc, push_chain(kproj_chain_ops(0, sc))))
        for j in range(8):
            v_need[j] = push_chain(vproj_chain_ops(j))
            checkpoints.append((j + 3, v_need[j]))
        # qp(0,1) rides mid-v so its iter-16 deadline doesn't force a burst
        checkpoints.append((block_gi[(0, 1)], push_chain(qproj_chain_ops(0, 1))))
        for j in range(8, NJ):
            v_need[j] = push_chain(vproj_chain_ops(j))
            checkpoints.append((j + 3, v_need[j]))
        checkpoints.append((block_gi[(0, 2)], push_chain(qproj_chain_ops(0, 2))))
        # qp(hp,0) early so qTs0's buffer frees for qTs3
        for hp in range(1, NHP):
            checkpoints.append((block_gi[(hp, 0)], push_chain(qproj_chain_ops(hp, 0))))
        checkpoints.append((block_gi[(0, 3)], push_chain(qproj_chain_ops(0, 3))))
        for hp in range(1, NHP):
            for sc in range(NSC):
                cp = push_chain(kproj_chain_ops(hp, sc))
            checkpoints.append((block_gi[(hp, 0)], cp))
            for sc in range(1, NSC):
                checkpoints.append((block_gi[(hp, sc)], push_chain(qproj_chain_ops(hp, sc))))
        FC_DEADLINE = 252

        def pace(gi):
            """Drain fillers: meet every checkpoint, else eagerly ~2/iter."""
            need = 0.0
            for it, cum in checkpoints:
                if it <= gi:
                    drain_until(cum)
                elif cum > drained[0]:
                    need = max(need, (cum - drained[0]) / (it - gi))
            backlog = emitted[0] - drained[0]
            if backlog > 0:
                need = max(
                    need,
                    backlog / max(1.0, FC_DEADLINE - gi),
                    min(2.0, float(backlog)),
                )
            debt[0] += need
            n = int(debt[0])
            if n > 0:
                drain(n)
                debt[0] -= n

        # ---- upfront: only what the first 4 iterations need ----
        for op in kproj_chain_ops(0, 0):
            op()
        for op in qproj_chain_ops(0, 0):
            op()

        # ---- attention blocks ----
        carry = []
        LAG = 3

        def attn_block(hp, qc, gi0):
            qs = slice(qc * 512, (qc + 1) * 512)
            state = {}
            p_tiles = {}

            def emit_st(j):
                ks = slice(j * P, (j + 1) * P)
                st = st_ps.tile([P, 1024], F32, tag="stps")
                nc.tensor.matmul(
                    st[:, 0:512],
                    lhsT=khT_sb[hp][0:64, ks],
                    rhs=qhT_sb[hp][0:64, qs],
                    start=True,
                    stop=True,
                    tile_position=(0, 0),
                )
                nc.tensor.matmul(
                    st[:, 512:1024],
                    lhsT=khT_sb[hp][64:128, ks],
                    rhs=qhT_sb[hp][64:128, qs],
                    start=True,
                    stop=True,
                    tile_position=(64, 0),
                )
                p = p_pool.tile([P, 1024], BF, tag="p")
                nc.scalar.activation(p[:], st[:], Exp, scale=SCALE)
                p_tiles[j] = p

            def emit_pv(j):
                if "P" not in state:
                    state["P"] = pv_ps.tile([P, 512], F32, tag="pvps", name="Pps")
                    state["L"] = pv_ps.tile([P, 512], F32, tag="pvps", name="Lps")
                P_ps, L_ps = state["P"], state["L"]
                p = p_tiles.pop(j)
                first, last = (j == 0), (j == NJ - 1)
                nc.tensor.matmul(
                    P_ps[0:64, :],
                    lhsT=vh_sb[j][:, hp * P : hp * P + DK],
                    rhs=p[:, 0:512],
                    start=first,
                    stop=last,
                    tile_position=(0, 0),
                    skip_group_check=True,
                )
                nc.tensor.matmul(
                    P_ps[64:128, :],
                    lhsT=vh_sb[j][:, hp * P + DK : (hp + 1) * P],
                    rhs=p[:, 512:1024],
                    start=first,
                    stop=last,
                    tile_position=(0, 64),
                    skip_group_check=True,
                )
                nc.tensor.matmul(
                    L_ps[0:64, :],
                    lhsT=ones_t[:],
                    rhs=p[:, 0:512],
                    start=first,
                    stop=last,
                    tile_position=(0, 0),
                    skip_group_check=True,
                )
                nc.tensor.matmul(
                    L_ps[64:128, :],
                    lhsT=ones_t[:],
                    rhs=p[:, 512:1024],
                    start=first,
                    stop=last,
                    tile_position=(0, 64),
                    skip_group_check=True,
                )

            for j in range(NJ):
                emit_st(j)
                # full drain at block start: frees the deferred p tiles
                # immediately (p_pool is sized for exactly this) and gets
                # normalize emitted before any fc chain can reference it
                while carry:
                    carry.pop(0)()
                if j == 4:
                    # prev block's carry (incl. normalize) has emitted by now,
                    # so fc chains reading its outT are safe to enqueue
                    while pending_fc:
                        push_chain(pending_fc.pop(0))
                if j >= LAG:
                    jj = j - LAG
                    drain_until(v_need.get(jj, 0))
                    emit_pv(jj)
                pace(gi0 + j)

            def mk_pv(j):
                def op():
                    drain_until(v_need.get(j, 0))
                    emit_pv(j)
                return op

            def normalize():
                rec = rec_pool.tile([P, 512], F32, tag="rec")
                nc.vector.reciprocal_approx_fast(rec[:], state["L"][:])
                nc.vector.tensor_mul(outT_sb[hp][:, qs], state["P"][:], rec[:])

            return [mk_pv(j) for j in range(NJ - LAG, NJ)] + [normalize]

        pending_fc = []
        for bi, (hp, qc) in enumerate(BLOCKS):
            carry = attn_block(hp, qc, 16 * bi)
            if hp == NHP - 1:
                last = bi == len(BLOCKS) - 1
                for i, sc in enumerate(range(qc * 4, qc * 4 + 4)):
                    for ec in range(2):
                        # tail batch alternates into the now-idle st_ps pool
                        # so chains pipeline two-deep
                        pool = st_ps if (last and (i * 2 + ec) % 2) else None
                        pending_fc.append(fc_chain_ops(sc, ec, pool))
        for op in carry:
            op()

        while pending_fc:
            fillers.extend(pending_fc.pop(0))
        while fillers:
            fillers.popleft()()
            drained[0] += 1

    nc.compile()
    return nc


def _get_nc():
    if "nc" not in _CACHE:
        _CACHE["nc"] = _build_nc()
    return _CACHE["nc"]


def kernel(q, k, v, Wq, bq, Wk, bk, Wv, bv, Wo, bo):
    from concourse.bass_utils import run_bass_kernel_spmd

    bf16 = ml_dtypes.bfloat16
    q, k, v = (np.asarray(x, np.float32) for x in (q, k, v))
    Wq, bq, Wk, bk, Wv, bv, Wo, bo = (
        np.asarray(x, np.float32) for x in (Wq, bq, Wk, bk, Wv, bv, Wo, bo)
    )

    def slab(a):
        """(n*128, C) -> (128, n*C): partition-major, m-chunk-major columns."""
        R, C = a.shape
        n = R // 128
        return np.ascontiguousarray(
            a.reshape(n, 128, C).transpose(1, 0, 2).reshape(128, n * C)
        )

    in_maps = []
    for c in range(NCORES):
        b, t = c // 2, c % 2
        hs = slice(t * HD, (t + 1) * HD)
        qT = q[b].T.astype(bf16)
        kT = k[b].T.astype(bf16)
        vT = v[b].T.astype(bf16)
        im = {
            "kTa": slab(kT[0 : DM // 2]),
            "kTb": slab(kT[DM // 2 :]),
            "wq": slab(Wq[:, hs].astype(bf16)),
            "wk": slab(Wk[:, hs].astype(bf16)),
            "wv": slab(Wv[:, hs].astype(bf16)),
            "wo": slab(Wo[hs, :].astype(bf16)),
            "bq": np.ascontiguousarray(bq[hs]),
            "bk": np.ascontiguousarray(bk[hs]),
        }
        for s in range(NSL):
            im[f"qTs{s}"] = slab(qT[:, s * 512 : (s + 1) * 512])
        for j in range(NJ):
            im[f"vTs{j}"] = slab(vT[:, j * 128 : (j + 1) * 128])
        in_maps.append(im)

    nc = _get_nc()
    trace = os.environ.get("KERNEL_TRACE", "0") == "1"
    res = run_bass_kernel_spmd(
        nc, in_maps, core_ids=list(range(NCORES)), trace=trace
    )
    if trace:
        print(f"HW exec time: {res.exec_time_ns} ns")

    host_bias = (bv @ Wo + bo).astype(np.float32)
    full = np.empty((NB, S, DM), np.float32)
    for b in range(NB):
        full[b] = res.results[2 * b]["out"] + res.results[2 * b + 1]["out"] + host_bias
    return full


# revision 25
# speedup vs baseline: 1.2135x; 1.1949x over previous
"""Multi-head attention (B=4, S=2048, D=1024, 16 heads x 64) on 8 NeuronCores.

Sharding: DP=4 over batch x TP=2 over heads (8 heads/core).
Each core computes, for one batch element and half the heads:
    qhT = (q @ Wq + bq)^T       [512, 2048]   (bf16, head-dim on partitions)
    khT = (k @ Wk + bk)^T       [512, 2048]
    vh  = v @ Wv                [2048, 512]   (natural layout, k on partitions)
    per (head-pair, q-chunk): flash-style S^T = kh @ qh^T, p = exp(scale*S^T),
      outT_u = vh^T @ p (col-tiled pair) and l = ones^T @ p (replicated rows),
      outT = outT_u * approx_recip(l)
    partial_out = outT^T @ Wo_shard          [2048, 1024]  (fp32)
Host sums the TP pair partials and adds the bias terms (bv @ Wo + bo).

v3 schedule: bulk DMA (one descriptor per tensor slab), attention starts as
soon as kT/qT-slice-0 land; v/k/q projection chains and the fc output
projection ride as deadline-paced fillers inside the attention iterations;
hp=2/hp=3 blocks interleave so fc work spreads instead of cliffing at the
end.  All PSUM drains off the Scalar engine (it only runs exp).
"""

import os
import sys

sys.path.insert(0, "/opt/trn_rl_repo")

import numpy as np
import ml_dtypes

S = 2048          # sequence length
DM = 1024         # model dim
HD = 512          # local head-dim total (8 heads x 64) per core (TP=2)
NB = 4            # batch
NCORES = 8
P = 128
DK = 64
SCALE = 1.0 / 8.0  # 1/sqrt(64)

NM = DM // P      # 8 m-chunks
NHP = HD // P     # 4 head pairs
NSC = S // 512    # 4 s-chunks of 512
NJ = S // P       # 16 k-chunks
NSL = 4           # input column slices of 512

_CACHE = {}


def _build_nc():
    import concourse.bass as bass  # noqa: F401
    import concourse.mybir as mybir
    from concourse import bacc, tile
    from contextlib import ExitStack

    BF = mybir.dt.bfloat16
    F32 = mybir.dt.float32
    Exp = mybir.ActivationFunctionType.Exp

    nc = bacc.Bacc("TRN2", target_bir_lowering=False, debug=False, num_swdge_queues=4)

    # DRAM inputs, pre-slabbed on the host into the exact SBUF layout
    # ([128 partitions, m-major columns]) so every DMA is one contiguous
    # full-rate 2D copy.
    kTa = nc.dram_tensor("kTa", [P, 4 * S], BF, kind="ExternalInput")
    kTb = nc.dram_tensor("kTb", [P, 4 * S], BF, kind="ExternalInput")
    qTs = [nc.dram_tensor(f"qTs{s}", [P, NM * 512], BF, kind="ExternalInput") for s in range(NSL)]
    vTs = [nc.dram_tensor(f"vTs{s}", [P, NM * P], BF, kind="ExternalInput") for s in range(NJ)]
    wq = nc.dram_tensor("wq", [P, NM * HD], BF, kind="ExternalInput")
    wk = nc.dram_tensor("wk", [P, NM * HD], BF, kind="ExternalInput")
    wv = nc.dram_tensor("wv", [P, NM * HD], BF, kind="ExternalInput")
    wo = nc.dram_tensor("wo", [P, NHP * DM], BF, kind="ExternalInput")
    bq = nc.dram_tensor("bq", [HD], F32, kind="ExternalInput")
    bk = nc.dram_tensor("bk", [HD], F32, kind="ExternalInput")
    out = nc.dram_tensor("out", [S, DM], F32, kind="ExternalOutput")

    with ExitStack() as ctx:
        tc = ctx.enter_context(tile.TileContext(nc))

        const = ctx.enter_context(tc.tile_pool(name="const", bufs=1))
        w_pool = ctx.enter_context(tc.tile_pool(name="w_pool", bufs=4))
        kt_pool = ctx.enter_context(tc.tile_pool(name="kt_pool", bufs=2))
        qt_pool = ctx.enter_context(tc.tile_pool(name="qt_pool", bufs=3))
        vt_pool = ctx.enter_context(tc.tile_pool(name="vt_pool", bufs=4))
        qh_pool = ctx.enter_context(tc.tile_pool(name="qh_pool", bufs=4))
        kh_pool = ctx.enter_context(tc.tile_pool(name="kh_pool", bufs=4))
        vh_pool = ctx.enter_context(tc.tile_pool(name="vh_pool", bufs=16))
        outT_pool = ctx.enter_context(tc.tile_pool(name="outT_pool", bufs=4))
        p_pool = ctx.enter_context(tc.tile_pool(name="p_pool", bufs=4))
        rec_pool = ctx.enter_context(tc.tile_pool(name="rec_pool", bufs=1))
        stage_pool = ctx.enter_context(tc.tile_pool(name="stage_pool", bufs=2))
        st_ps = ctx.enter_context(tc.tile_pool(name="st_ps", bufs=2, space="PSUM"))
        pv_ps = ctx.enter_context(tc.tile_pool(name="pv_ps", bufs=2, space="PSUM"))
        wk_ps = ctx.enter_context(tc.tile_pool(name="wk_ps", bufs=2, space="PSUM"))

        # constants (gpsimd queue)
        ones_t = const.tile([P, DK], BF, tag="ones")
        nc.vector.memset(ones_t[:], 1.0)
        bq_sb = const.tile([P, NHP], F32, tag="bq")
        nc.gpsimd.dma_start(bq_sb[:], bq[:].rearrange("(f p) -> p f", p=P))
        bk_sb = const.tile([P, NHP], F32, tag="bk")
        nc.gpsimd.dma_start(bk_sb[:], bk[:].rearrange("(f p) -> p f", p=P))

        # ---- bulk input DMA, priority-ordered on the sync queue ----
        # Slabs are host-pre-arranged: each is one contiguous 2D copy.
        def load_slab(pool, handle, tag):
            t = pool.tile([P, handle.shape[1]], BF, tag=tag)
            nc.sync.dma_start(t[:], handle[:, :])
            return t

        wk_sb = load_slab(w_pool, wk, "w")       # [128, 8*512]
        wq_sb = load_slab(w_pool, wq, "w")
        qT_sb = [None] * NSL
        vT_sb = [None] * NJ
        qT_sb[0] = load_slab(qt_pool, qTs[0], "qt")   # [128, 8*512]
        kT_a = load_slab(kt_pool, kTa, "kt")
        kT_b = load_slab(kt_pool, kTb, "kt")
        wv_sb = load_slab(w_pool, wv, "w")
        for j in range(4):
            vT_sb[j] = load_slab(vt_pool, vTs[j], "vt")
        qT_sb[1] = load_slab(qt_pool, qTs[1], "qt")
        for j in range(4, 8):
            vT_sb[j] = load_slab(vt_pool, vTs[j], "vt")
        qT_sb[2] = load_slab(qt_pool, qTs[2], "qt")
        for j in range(8, NJ):
            vT_sb[j] = load_slab(vt_pool, vTs[j], "vt")
        qT_sb[3] = load_slab(qt_pool, qTs[3], "qt")
        wo_sb = load_slab(w_pool, wo, "wo")      # [128, 4*1024]

        # ---- PE warm-up: keep the HAM clock gate open during the DMA
        # prefix so the first projection chains run at 2.4 GHz ----
        warm_p = p_pool.tile([P, 1024], BF, tag="p", name="warm")
        nc.vector.memset(warm_p[:], 0.0)
        wps = wk_ps.tile([P, HD], F32, tag="wps", name="warm_ps")
        for _ in range(38):
            nc.tensor.matmul(
                wps[0:64, :], lhsT=ones_t[:], rhs=warm_p[:, 0:512], start=True, stop=True
            )

        def w_sl(t, m, lo, hi, w=512):
            return t[:, m * w + lo : m * w + hi]

        def kt_sl(m, lo, hi):
            t = kT_a if m < 4 else kT_b
            return t[:, (m % 4) * S + lo : (m % 4) * S + hi]

        # ---- persistent activation tiles ----
        qhT_sb = [qh_pool.tile([P, S], BF, tag="qh", name=f"qhT{i}") for i in range(NHP)]
        khT_sb = [kh_pool.tile([P, S], BF, tag="kh", name=f"khT{i}") for i in range(NHP)]
        outT_sb = [outT_pool.tile([P, S], BF, tag="outT", name=f"outT{i}") for i in range(NHP)]
        vh_sb = [vh_pool.tile([P, HD], BF, tag="vh", name=f"vh{i}") for i in range(NJ)]

        # ---- chain builders (lists of single-op closures) ----
        def vproj_chain_ops(sc):
            """vh[sc] = vT[:, sc-chunk]^T @ wv   (natural [s, hd] layout)."""
            cell = {}

            def mk(m):
                def op():
                    if m == 0:
                        cell["ps"] = wk_ps.tile([P, HD], F32, tag="wps", name="vps")
                    nc.tensor.matmul(
                        cell["ps"][:],
                        lhsT=w_sl(vT_sb[sc], m, 0, P, w=P),
                        rhs=w_sl(wv_sb, m, 0, HD),
                        start=(m == 0),
                        stop=(m == NM - 1),
                    )
                return op

            ops = [mk(m) for m in range(NM)]
            ops.append(lambda: nc.vector.tensor_copy(vh_sb[sc][:], cell["ps"][:]))
            return ops

        def qproj_chain_ops(hp, sc):
            cell = {}

            def mk(m):
                def op():
                    if m == 0:
                        cell["ps"] = wk_ps.tile([P, HD], F32, tag="wps", name="qps")
                    nc.tensor.matmul(
                        cell["ps"][:],
                        lhsT=w_sl(wq_sb, m, hp * P, (hp + 1) * P),
                        rhs=w_sl(qT_sb[sc], m, 0, 512),
                        start=(m == 0),
                        stop=(m == NM - 1),
                    )
                return op

            ops = [mk(m) for m in range(NM)]

            def ev():
                nc.vector.tensor_scalar_add(
                    qhT_sb[hp][:, sc * 512 : (sc + 1) * 512],
                    cell["ps"][:],
                    bq_sb[:, hp : hp + 1],
                )

            ops.append(ev)
            return ops

        def kproj_chain_ops(hp, sc):
            cell = {}

            def mk(m):
                def op():
                    if m == 0:
                        cell["ps"] = wk_ps.tile([P, HD], F32, tag="wps", name="kps")
                    nc.tensor.matmul(
                        cell["ps"][:],
                        lhsT=w_sl(wk_sb, m, hp * P, (hp + 1) * P),
                        rhs=kt_sl(m, sc * 512, (sc + 1) * 512),
                        start=(m == 0),
                        stop=(m == NM - 1),
                    )
                return op

            ops = [mk(m) for m in range(NM)]

            def ev():
                nc.vector.tensor_scalar_add(
                    khT_sb[hp][:, sc * 512 : (sc + 1) * 512],
                    cell["ps"][:],
                    bk_sb[:, hp : hp + 1],
                )

            ops.append(ev)
            return ops

        def fc_chain_ops(sc, ec, pool=None):
            ss = slice(sc * P, (sc + 1) * P)
            cell = {}

            def mk(hp):
                def op():
                    if hp == 0:
                        pl = pool or wk_ps
                        cell["ps"] = pl.tile(
                            [P, 1024 if pl is st_ps else HD], F32,
                            tag="stps" if pl is st_ps else "wps", name="fps",
                        )[:, 0:HD]
                    nc.tensor.matmul(
                        cell["ps"][:],
                        lhsT=outT_sb[hp][:, ss],
                        rhs=w_sl(wo_sb, hp, ec * 512, (ec + 1) * 512, w=DM),
                        start=(hp == 0),
                        stop=(hp == NHP - 1),
                    )
                return op

            ops = [mk(hp) for hp in range(NHP)]

            def ev():
                stg = stage_pool.tile([P, 512], F32, tag="stg", name="stg")
                if pool is st_ps:
                    # tail batch: ACT is idle (no more exps) and alternating
                    # engines dodges the DVE drain-serialization
                    nc.scalar.copy(stg[:], cell["ps"][:])
                else:
                    nc.vector.tensor_copy(stg[:], cell["ps"][:])
                nc.gpsimd.dma_start(out[ss, ec * 512 : (ec + 1) * 512], stg[:])

            ops.append(ev)
            return ops

        # ---- filler queue with deadline pacing ----
        from collections import deque

        fillers = deque()   # flat op deque
        drained = [0]       # ops popped so far
        emitted = [0]       # ops pushed so far
        checkpoints = []    # (global_iter, cum_ops needed by then)
        debt = [0.0]

        def push_chain(ops):
            fillers.extend(ops)
            emitted[0] += len(ops)
            return emitted[0]

        def drain(n):
            for _ in range(n):
                if not fillers:
                    return
                fillers.popleft()()
                drained[0] += 1

        def drain_until(cum):
            while drained[0] < cum and fillers:
                fillers.popleft()()
                drained[0] += 1

        # block order: hp 0 and 1 sweep qc; then hp2/hp3 interleave per qc so
        # fc(qc) (which needs all four hp at qc) unlocks early and spreads.
        BLOCKS = (
            [(0, qc) for qc in range(NSC)]
            + [(1, qc) for qc in range(NSC)]
            + [b for qc in range(NSC) for b in ((2, qc), (3, qc))]
        )
        block_gi = {b: 16 * i for i, b in enumerate(BLOCKS)}

        # filler order: v chains first (PV gating), then per-hp k/q proj in
        # first-use order.
        v_need = {}
        # kp0 sc1-3: needed by st(j>=4) of block (0,0) - fine-grained deadlines
        for sc in range(1, NSC):
            checkpoints.append((4 * sc, push_chain(kproj_chain_ops(0, sc))))
        for j in range(8):
            v_need[j] = push_chain(vproj_chain_ops(j))
            checkpoints.append((j + 3, v_need[j]))
        # qp(0,1) rides mid-v so its iter-16 deadline doesn't force a burst
        checkpoints.append((block_gi[(0, 1)], push_chain(qproj_chain_ops(0, 1))))
        for j in range(8, NJ):
            v_need[j] = push_chain(vproj_chain_ops(j))
            checkpoints.append((j + 3, v_need[j]))
        checkpoints.append((block_gi[(0, 2)], push_chain(qproj_chain_ops(0, 2))))
        # qp(hp,0) early so qTs0's buffer frees for qTs3
        for hp in range(1, NHP):
            checkpoints.append((block_gi[(hp, 0)], push_chain(qproj_chain_ops(hp, 0))))
        checkpoints.append((block_gi[(0, 3)], push_chain(qproj_chain_ops(0, 3))))
        for hp in range(1, NHP):
            for sc in range(NSC):
                cp = push_chain(kproj_chain_ops(hp, sc))
            checkpoints.append((block_gi[(hp, 0)], cp))
            for sc in range(1, NSC):
                checkpoints.append((block_gi[(hp, sc)], push_chain(qproj_chain_ops(hp, sc))))
        FC_DEADLINE = 252

        def pace(gi):
            """Drain fillers: meet every checkpoint, else eagerly ~2/iter."""
            need = 0.0
            for it, cum in checkpoints:
                if it <= gi:
                    drain_until(cum)
                elif cum > drained[0]:
                    need = max(need, (cum - drained[0]) / (it - gi))
            backlog = emitted[0] - drained[0]
            if backlog > 0:
                need = max(
                    need,
                    backlog / max(1.0, FC_DEADLINE - gi),
                    min(2.0, float(backlog)),
                )
            debt[0] += need
            n = int(debt[0])
            if n > 0:
                drain(n)
                debt[0] -= n

        # ---- upfront: only what the first 4 iterations need; qp00 first
        # since qTs0 lands before the kT slabs ----
        for op in qproj_chain_ops(0, 0):
            op()
        for op in kproj_chain_ops(0, 0):
            op()

        # ---- attention blocks ----
        carry = []
        LAG = 3

        def attn_block(hp, qc, gi0):
            qs = slice(qc * 512, (qc + 1) * 512)
            state = {}
            p_tiles = {}

            def emit_st(j):
                ks = slice(j * P, (j + 1) * P)
                st = st_ps.tile([P, 1024], F32, tag="stps")
                nc.tensor.matmul(
                    st[:, 0:512],
                    lhsT=khT_sb[hp][0:64, ks],
                    rhs=qhT_sb[hp][0:64, qs],
                    start=True,
                    stop=True,
                    tile_position=(0, 0),
                )
                nc.tensor.matmul(
                    st[:, 512:1024],
                    lhsT=khT_sb[hp][64:128, ks],
                    rhs=qhT_sb[hp][64:128, qs],
                    start=True,
                    stop=True,
                    tile_position=(64, 0),
                )
                p = p_pool.tile([P, 1024], BF, tag="p")
                nc.scalar.activation(p[:], st[:], Exp, scale=SCALE)
                p_tiles[j] = p

            def emit_pv(j):
                if "P" not in state:
                    state["P"] = pv_ps.tile([P, 512], F32, tag="pvps", name="Pps")
                    state["L"] = pv_ps.tile([P, 512], F32, tag="pvps", name="Lps")
                P_ps, L_ps = state["P"], state["L"]
                p = p_tiles.pop(j)
                first, last = (j == 0), (j == NJ - 1)
                nc.tensor.matmul(
                    P_ps[0:64, :],
                    lhsT=vh_sb[j][:, hp * P : hp * P + DK],
                    rhs=p[:, 0:512],
                    start=first,
                    stop=last,
                    tile_position=(0, 0),
                    skip_group_check=True,
                )
                nc.tensor.matmul(
                    P_ps[64:128, :],
                    lhsT=vh_sb[j][:, hp * P + DK : (hp + 1) * P],
                    rhs=p[:, 512:1024],
                    start=first,
                    stop=last,
                    tile_position=(0, 64),
                    skip_group_check=True,
                )
                nc.tensor.matmul(
                    L_ps[0:64, :],
                    lhsT=ones_t[:],
                    rhs=p[:, 0:512],
                    start=first,
                    stop=last,
                    tile_position=(0, 0),
                    skip_group_check=True,
                )
                nc.tensor.matmul(
                    L_ps[64:128, :],
                    lhsT=ones_t[:],
                    rhs=p[:, 512:1024],
                    start=first,
                    stop=last,
                    tile_position=(0, 64),
                    skip_group_check=True,
                )

            for j in range(NJ):
                emit_st(j)
                # full drain at block start: frees the deferred p tiles
                # immediately (p_pool is sized for exactly this) and gets
                # normalize emitted before any fc chain can reference it
                while carry:
                    carry.pop(0)()
                if j == 4:
                    # prev block's carry (incl. normalize) has emitted by now,
                    # so fc chains reading its outT are safe to enqueue
                    while pending_fc:
                        push_chain(pending_fc.pop(0))
                if j >= LAG:
                    jj = j - LAG
                    drain_until(v_need.get(jj, 0))
                    emit_pv(jj)
                pace(gi0 + j)

            def mk_pv(j):
                def op():
                    drain_until(v_need.get(j, 0))
                    emit_pv(j)
                return op

            def normalize():
                rec = rec_pool.tile([P, 512], F32, tag="rec")
                nc.vector.reciprocal_approx_fast(rec[:], state["L"][:])
                nc.vector.tensor_mul(outT_sb[hp][:, qs], state["P"][:], rec[:])

            return [mk_pv(j) for j in range(NJ - LAG, NJ)] + [normalize]

        pending_fc = []
        for bi, (hp, qc) in enumerate(BLOCKS):
            carry = attn_block(hp, qc, 16 * bi)
            if hp == NHP - 1:
                last = bi == len(BLOCKS) - 1
                for i, sc in enumerate(range(qc * 4, qc * 4 + 4)):
                    for ec in range(2):
                        # tail batch alternates into the now-idle st_ps pool
                        # so chains pipeline two-deep
                        pool = st_ps if (last and (i * 2 + ec) % 2) else None
                        pending_fc.append(fc_chain_ops(sc, ec, pool))
        for op in carry:
            op()

        while pending_fc:
            fillers.extend(pending_fc.pop(0))
        while fillers:
            fillers.popleft()()
            drained[0] += 1

    nc.compile()
    return nc


def _get_nc():
    if "nc" not in _CACHE:
        _CACHE["nc"] = _build_nc()
    return _CACHE["nc"]


def kernel(q, k, v, Wq, bq, Wk, bk, Wv, bv, Wo, bo):
    from concourse.bass_utils import run_bass_kernel_spmd

    bf16 = ml_dtypes.bfloat16
    q, k, v = (np.asarray(x, np.float32) for x in (q, k, v))
    Wq, bq, Wk, bk, Wv, bv, Wo, bo = (
        np.asarray(x, np.float32) for x in (Wq, bq, Wk, bk, Wv, bv, Wo, bo)
    )

    def slab(a):
        """(n*128, C) -> (128, n*C): partition-major, m-chunk-major columns."""
        R, C = a.shape
        n = R // 128
        return np.ascontiguousarray(
            a.reshape(n, 128, C).transpose(1, 0, 2).reshape(128, n * C)
        )

    in_maps = []
    for c in range(NCORES):
        b, t = c // 2, c % 2
        hs = slice(t * HD, (t + 1) * HD)
        qT = q[b].T.astype(bf16)
        kT = k[b].T.astype(bf16)
        vT = v[b].T.astype(bf16)
        im = {
            "kTa": slab(kT[0 : DM // 2]),
            "kTb": slab(kT[DM // 2 :]),
            "wq": slab(Wq[:, hs].astype(bf16)),
            "wk": slab(Wk[:, hs].astype(bf16)),
            "wv": slab(Wv[:, hs].astype(bf16)),
            "wo": slab(Wo[hs, :].astype(bf16)),
            "bq": np.ascontiguousarray(bq[hs]),
            "bk": np.ascontiguousarray(bk[hs]),
        }
        for s in range(NSL):
            im[f"qTs{s}"] = slab(qT[:, s * 512 : (s + 1) * 512])
        for j in range(NJ):
            im[f"vTs{j}"] = slab(vT[:, j * 128 : (j + 1) * 128])
        in_maps.append(im)

    nc = _get_nc()
    trace = os.environ.get("KERNEL_TRACE", "0") == "1"
    res = run_bass_kernel_spmd(
        nc, in_maps, core_ids=list(range(NCORES)), trace=trace
    )
    if trace:
        print(f"HW exec time: {res.exec_time_ns} ns")

    host_bias = (bv @ Wo + bo).astype(np.float32)
    full = np.empty((NB, S, DM), np.float32)
    for b in range(NB):
        full[b] = res.results[2 * b]["out"] + res.results[2 * b + 1]["out"] + host_bias
    return full
